# revision 9
# baseline (speedup 1.0000x reference)
"""Trainium2 Bass kernel for nn_CustomNodeGCN (GCN message passing).

Graph-parallel across 8 NeuronCores; per conv layer: per-core shard
u = dinv*(h@W) -> AllGather the bf16 row table -> dma_gather edge
messages into padded per-dst slots -> DVE segment-sum -> BN/ReLU.
Key measured optimizations over the first working version:
  - dma_gather is descriptor-rate bound: 8-deep gather buffer pool x 4
    SWDGE queues keeps ~49k rows in flight (2.5x gather throughput).
  - Balanced 4/4 core-group gather windows + octet-greedy per-dst
    window balancing (cuts K padding).
  - Self-loop term dinv*u added on-chip (node-major shard tile is kept
    in SBUF), removing ~6% of gathered rows.
  - All 21 kernel inputs packed into one f32 tensor (idx bitcast into
    it): each extra PJRT binding costs ~33us/iteration.
  - Shared->local table copy split per window so window-A gathers
    start after half the copy.
"""

import math
import os

import numpy as np

# ---------------------------------------------------------------- config ----
N_NODES = 50000
E_EDGES = 800000
DIN = 128
H = 128
DOUT = 64
EPS = 1e-5

C = 8          # cores
P = 128        # partitions
A_CORES = 4    # cores 0..3 feed gather window A; 4..7 feed window B
GROUP_CAP = int(os.environ.get("GCN_GROUP_CAP", "48"))
GBUFS = int(os.environ.get("GCN_GBUFS", "8"))

_cache = {}


# ---------------------------------------------------------- preprocessing ---
def _preprocess(edge_index, n_nodes):
    src = edge_index[0].astype(np.int64)
    dst = edge_index[1].astype(np.int64)
    N = n_nodes
    assert N % C == 0
    REAL = N // C
    TPC = REAL // P + 1          # tiles per core (>=1 dummy slot per core)
    SL = TPC * P                 # slots per core
    T_ROWS = C * SL
    WB_BASE = A_CORES * SL
    assert A_CORES * SL <= 32768, "window A must fit int16 addressing"
    assert T_ROWS - WB_BASE <= 32768, "window B must fit int16 addressing"

    deg = np.bincount(dst, minlength=N) + 1           # in-degree + self loop
    dinv = 1.0 / np.sqrt(deg.astype(np.float64))

    # core assignment: deal by total degree rank, then swap within each
    # rank octet to balance every dst's window-A/window-B in-degree
    order0 = np.argsort(deg, kind="stable")
    rank0 = np.empty(N, np.int64)
    rank0[order0] = np.arange(N)
    core = rank0 % C

    # greedy octet balancing (vote by current dst imbalance)
    out_src = np.argsort(src, kind="stable")
    out_start = np.searchsorted(src[out_src], np.arange(N + 1))
    b = np.zeros(N, np.float64)   # dst imbalance dA - dB
    groupA = np.zeros(N, bool)
    octets = order0.reshape(-1, C)    # each row: 8 nodes w/ adjacent ranks
    for row in octets:
        prefs = []
        for u in row:
            es = out_src[out_start[u]:out_start[u + 1]]
            vote = b[dst[es]].sum() + b[u]      # + self loop
            prefs.append(vote)
        sel = np.argsort(np.asarray(prefs), kind="stable")[:C // 2]
        amask = np.zeros(C, bool)
        amask[sel] = True
        for j, u in enumerate(row):
            groupA[u] = amask[j]
            d = 1.0 if amask[j] else -1.0
            es = out_src[out_start[u]:out_start[u + 1]]
            np.add.at(b, dst[es], d)
            b[u] += d
    # core: group-A nodes -> cores 0..3, group-B -> 4..7, keeping the
    # degree-graded deal within each group
    core = np.empty(N, np.int64)
    acnt = 0
    bcnt = 0
    for row in octets:
        for u in row:
            if groupA[u]:
                core[u] = acnt % A_CORES
                acnt += 1
            else:
                core[u] = A_CORES + bcnt % (C - A_CORES)
                bcnt += 1
    assert acnt == N // 2 and bcnt == N // 2, (acnt, bcnt)

    # per-window in-degree (self loops handled on-chip, not gathered)
    srcA = groupA[src]
    dA = np.bincount(dst[srcA], minlength=N)
    dB = np.bincount(dst[~srcA], minlength=N)

    # within-core ordering: (dA asc, dB desc) packs tiles tightly
    local = np.empty(N, np.int64)
    for c in range(C):
        idx = np.where(core == c)[0]
        key = dA[idx] * 100000 - dB[idx]
        o = idx[np.argsort(key, kind="stable")]
        local[o] = np.arange(len(o))
    row = core * SL + local      # table row of each node

    # per-tile K (max over cores => same program on every core)
    KA = np.zeros((C, TPC), np.int64)
    KB = np.zeros((C, TPC), np.int64)
    for c in range(C):
        m = core == c
        t = local[m] // P
        np.maximum.at(KA[c], t, dA[m])
        np.maximum.at(KB[c], t, dB[m])
    KAg = KA.max(0)
    KBg = KB.max(0)

    # gather call groups: consecutive tiles, capped K sum
    groups = []
    g = []
    ksum = 0
    for t in range(TPC):
        kt = int(KAg[t] + KBg[t])
        if g and ksum + kt > GROUP_CAP:
            groups.append(g)
            g, ksum = [], 0
        g.append(t)
        ksum += kt
    if g:
        groups.append(g)

    baseA = np.zeros(TPC, np.int64)
    baseB = np.zeros(TPC, np.int64)
    gmeta = []                        # (rowA0, nA_rows, rowB0, nB_rows)
    cur = 0
    for g in groups:
        a0 = cur
        for t in g:
            baseA[t] = cur
            cur += KAg[t]
        b0 = cur
        for t in g:
            baseB[t] = cur
            cur += KBg[t]
        gmeta.append((a0, b0 - a0, b0, cur - b0))
    TOTK = cur
    TOT_SLOTS = TOTK * P

    padA_row = 0 * SL + REAL          # core 0's first dummy slot (zero row)
    padB_row = A_CORES * SL + REAL
    assert padA_row <= 32767

    xt_perm = np.zeros((C, SL), np.int64)
    valid = np.zeros((C, SL), bool)
    for c in range(C):
        m = np.where(core == c)[0]
        xt_perm[c, local[m]] = m
        valid[c, local[m]] = True
    assert (valid[:, :REAL]).all() and not valid[:, REAL:].any()

    dinv_cols = np.zeros((C, P, TPC), np.float32)
    for c in range(C):
        loc = local[core == c]
        nodes = np.where(core == c)[0]
        dinv_cols[c, loc % P, loc // P] = dinv[nodes]

    idx16 = np.zeros((C, TOT_SLOTS), np.int16)
    padA_val = np.int16(padA_row)
    padB_val = np.int16(padB_row - WB_BASE)
    for (a0, na, b0, nb) in gmeta:
        idx16[:, a0 * P:(a0 + na) * P] = padA_val
        idx16[:, b0 * P:(b0 + nb) * P] = padB_val

    e_src = src
    e_dst = dst
    e_srow = row[e_src]
    e_A = groupA[e_src]
    e_c = core[e_dst]
    e_loc = local[e_dst]
    okey = e_c * (SL * 2) + e_loc * 2 + (~e_A)
    eo = np.argsort(okey, kind="stable")
    sk = okey[eo]
    first = np.r_[True, sk[1:] != sk[:-1]]
    starts = np.where(first)[0]
    grp = np.cumsum(first) - 1
    k_in_grp = np.arange(len(eo)) - starts[grp]
    ks = np.empty(len(eo), np.int64)
    ks[eo] = k_in_grp

    t_of = e_loc // P
    p_of = e_loc % P
    base = np.where(e_A, baseA[t_of], baseB[t_of])
    slot = (base + ks) * P + p_of
    val = np.where(e_A, e_srow, e_srow - WB_BASE).astype(np.int16)
    idx16[e_c, slot] = val

    idx_sb = idx16.reshape(C, TOT_SLOTS // 16, 16).transpose(0, 2, 1)
    idx_sb = np.tile(idx_sb, (1, 8, 1)).copy()

    return dict(
        REAL=REAL, TPC=TPC, SL=SL, T_ROWS=T_ROWS, WB_BASE=WB_BASE,
        KAg=KAg, KBg=KBg, groups=groups, gmeta=gmeta,
        baseA=baseA, baseB=baseB, TOTK=TOTK,
        xt_perm=xt_perm, dinv_cols=dinv_cols, idx_sb=idx_sb,
    )


# ------------------------------------------------------------- bass build ---
def _build(meta, n_real_total):
    import concourse.bacc as bacc
    import concourse.bass as bass
    import concourse.mybir as mybir
    import concourse.tile as tile
    from concourse.masks import make_identity

    f32 = mybir.dt.float32
    bf16 = mybir.dt.bfloat16
    i16 = mybir.dt.int16
    AF = mybir.ActivationFunctionType

    TPC, SL, T_ROWS = meta["TPC"], meta["SL"], meta["T_ROWS"]
    REAL = meta["REAL"]
    WB_BASE = meta["WB_BASE"]
    KAg, KBg = meta["KAg"], meta["KBg"]
    groups, gmeta = meta["groups"], meta["gmeta"]
    baseA, baseB = meta["baseA"], meta["baseB"]
    TOTK = meta["TOTK"]
    IDX_COLS = TOTK * P // 16
    WA_ROWS = WB_BASE
    WB_ROWS = T_ROWS - WB_BASE

    nc = bacc.Bacc("TRN2", debug=False, num_devices=C, num_swdge_queues=4)

    # all float inputs packed into one tensor (each extra kernel-I/O
    # binding costs ~33us/iteration through the PJRT launch path):
    # [ x_t(SL) | dinv(TPC) | 6x HxH weights | pw2(DOUT) | 9 bias cols ]
    w_names = ["pre_w1", "pre_w2", "cw0", "cw1", "cw2", "pw1"]
    v_names = ["pre_b1", "pre_b2", "cb2", "bng0", "bnb0", "bng1", "bnb1",
               "pb1", "pb2"]
    off = {}
    cur = 0
    for n, wdt in [("x_t", SL), ("dinv", TPC)] + \
                  [(n, H) for n in w_names] + [("pw2", DOUT)] + \
                  [(n, 1) for n in v_names]:
        off[n] = cur
        cur += wdt
    NF32 = cur
    assert IDX_COLS % 2 == 0
    big_in = nc.dram_tensor("big", [P, NF32 + IDX_COLS // 2], f32,
                            kind="ExternalInput")
    out_t = nc.dram_tensor("out_t", [DOUT, SL], f32, kind="ExternalOutput")

    def bslice(n, wdt):
        return big_in[:, off[n]:off[n] + wdt]

    idx_src = big_in[:, NF32:NF32 + IDX_COLS // 2].bitcast(i16)

    chunks = []
    o = 0
    while o < SL:
        w = min(512, SL - o)
        chunks.append((o, w))
        o += w

    with tile.TileContext(nc, num_cores=C) as tc:
        with (
            tc.tile_pool(name="persist", bufs=1) as pp,
            tc.tile_pool(name="gbuf", bufs=GBUFS) as gp,
            tc.tile_pool(name="work", bufs=3) as wp,
            tc.tile_pool(name="nodework", bufs=4) as nwp,
            tc.tile_pool(name="pmm", bufs=2, space="PSUM") as pmm,
            tc.tile_pool(name="ptp", bufs=4, space="PSUM") as ptp,
            tc.tile_pool(name="dram", bufs=1, space="DRAM") as dp,
        ):
            h_sb = pp.tile([P, SL], f32, tag="h")
            acc_sb = pp.tile([P, SL], f32, tag="acc")
            tn_all = pp.tile([P, SL], bf16, tag="tnall")  # dinv*u node-major
            idx_sb = pp.tile([P, IDX_COLS], i16, tag="idx")
            dinv_sb = pp.tile([P, TPC], f32, tag="dinv")
            ident = pp.tile([P, P], f32, tag="ident")
            w_sb = {n: pp.tile([H, H], f32, tag=f"w_{n}", name=f"w_{n}")
                    for n in w_names}
            w_sb["pw2"] = pp.tile([H, DOUT], f32, tag="w_pw2", name="w_pw2")
            v_sb = {n: pp.tile([H, 1], f32, tag=f"v_{n}", name=f"v_{n}")
                    for n in v_names}
            xt_sb = h_sb

            shard_d = dp.tile([SL, H], bf16, tag="shard")
            table_ds = [dp.tile([T_ROWS, H], bf16, tag=f"table{i}",
                                name=f"table{i}")
                        for i in range(3)]
            tableS_ds = [dp.tile([T_ROWS, H], bf16, tag=f"tableS{i}",
                                 name=f"tableS{i}", addr_space="Shared")
                         for i in range(3)]
            st_in_d = dp.tile([P, 2], f32, tag="stin")
            st_out_ds = [dp.tile([P, 2], f32, tag=f"stout{i}",
                                 name=f"stout{i}")
                         for i in range(2)]

            nc.sync.dma_start(xt_sb[:], bslice("x_t", SL))
            nc.sync.dma_start(idx_sb[:], idx_src)
            nc.sync.dma_start(dinv_sb[:], bslice("dinv", TPC))
            for n in w_names:
                nc.sync.dma_start(w_sb[n][:], bslice(n, H))
            nc.sync.dma_start(w_sb["pw2"][:], bslice("pw2", DOUT))
            for n in v_names:
                nc.sync.dma_start(v_sb[n][:], bslice(n, 1))
            make_identity(nc, ident[:])

            # ---- pre-MLP (feature-major) ----
            for (o, w) in chunks:
                ps = pmm.tile([P, 512], f32, space="PSUM", tag="mm")
                nc.tensor.matmul(ps[:, :w], lhsT=w_sb["pre_w1"][:],
                                 rhs=xt_sb[:, o:o + w], start=True, stop=True)
                t0 = wp.tile([P, 512], f32, tag="u512")
                nc.scalar.activation(t0[:, :w], ps[:, :w], AF.Relu,
                                     bias=v_sb["pre_b1"][:, 0:1])
                ps2 = pmm.tile([P, 512], f32, space="PSUM", tag="mm")
                nc.tensor.matmul(ps2[:, :w], lhsT=w_sb["pre_w2"][:],
                                 rhs=t0[:, :w], start=True, stop=True)
                nc.scalar.activation(h_sb[:, o:o + w], ps2[:, :w], AF.Relu,
                                     bias=v_sb["pre_b2"][:, 0:1])
            nc.vector.memset(h_sb[:, REAL:SL], 0.0)

            # ---- conv layers ----
            n_layers = int(os.environ.get("GCN_LAYERS", "3"))
            skip_bn = bool(os.environ.get("GCN_SKIP_BN"))
            skip_gather = bool(os.environ.get("GCN_SKIP_GATHER"))
            skip_coll = bool(os.environ.get("GCN_SKIP_COLL"))
            skip_reduce = bool(os.environ.get("GCN_SKIP_REDUCE"))
            zbuf = None
            if skip_gather:
                zbuf = pp.tile([P, GROUP_CAP, H], bf16, tag="zbuf")
                nc.vector.memset(zbuf[:], 0.0)
            layer_list = [("cw0", True), ("cw1", True), ("cw2", False)][:n_layers]
            for layer, (wn, has_bn) in enumerate(layer_list):
                has_bn = has_bn and not skip_bn
                for ci, (o, w) in enumerate(chunks):
                    ps = pmm.tile([P, 512], f32, space="PSUM", tag="mm")
                    nc.tensor.matmul(ps[:, :w], lhsT=w_sb[wn][:],
                                     rhs=h_sb[:, o:o + w], start=True, stop=True)
                    u0 = wp.tile([P, 512], f32, tag="u512")
                    nc.scalar.copy(u0[:, :w], ps[:, :w])
                    for b in range(w // P):
                        t = (o // P) + b
                        pt = ptp.tile([P, P], f32, space="PSUM", tag="tp")
                        nc.tensor.transpose(pt[:], u0[:, b * P:(b + 1) * P],
                                            ident[:])
                        tn = tn_all[:, t * P:(t + 1) * P]
                        nc.scalar.activation(tn, pt[:], AF.Copy,
                                             scale=dinv_sb[:, t:t + 1])
                        nc.sync.dma_start(shard_d[t * P:(t + 1) * P, :], tn)

                table_d = table_ds[layer]
                table_s = tableS_ds[layer]
                if skip_coll:
                    nc.sync.dma_start(table_s[0:SL, :], shard_d[:, :])
                else:
                    nc.gpsimd.collective_compute(
                        "AllGather", mybir.AluOpType.bypass,
                        replica_groups=[list(range(C))],
                        ins=[shard_d[:, :].opt()],
                        outs=[table_s[:, :].opt()],
                    )
                # split the Shared->local copy so window-A gathers start
                # after only half the copy
                nc.sync.dma_start(table_d[0:WA_ROWS, :],
                                  table_s[0:WA_ROWS, :])
                nc.sync.dma_start(table_d[WB_BASE:T_ROWS, :],
                                  table_s[WB_BASE:T_ROWS, :])

                tabA = table_d[0:WA_ROWS, :]
                tabB = table_d[WB_BASE:WB_BASE + WB_ROWS, :]
                qn = 0
                for gi, g in enumerate(groups):
                    a0, na, b0, nb = gmeta[gi]
                    gb = gp.tile([P, GROUP_CAP, H], bf16, tag="gather")
                    if skip_gather:
                        gb = zbuf
                    else:
                        if na:
                            nc.gpsimd.dma_gather(
                                gb[:, 0:na, :], tabA,
                                idx_sb[:, a0 * 8:(a0 + na) * 8],
                                na * P, na * P, H, single_packet=False,
                                queue_num=qn % 4)
                            qn += 1
                        if nb:
                            nc.gpsimd.dma_gather(
                                gb[:, na:na + nb, :], tabB,
                                idx_sb[:, b0 * 8:(b0 + nb) * 8],
                                nb * P, nb * P, H, single_packet=False,
                                queue_num=qn % 4)
                            qn += 1
                    for t in (() if skip_reduce else g):
                        ka, kb = int(KAg[t]), int(KBg[t])
                        oa = int(baseA[t] - a0)
                        ob = int(baseB[t] - a0)
                        tns = tn_all[:, t * P:(t + 1) * P]
                        accn = nwp.tile([P, P], f32, tag="accn")
                        if ka and kb:
                            wa = nwp.tile([P, P], f32, tag="redA")
                            nc.vector.reduce_sum(
                                wa[:], gb[:, oa:oa + ka, :].rearrange(
                                    "p k f -> p f k"),
                                axis=mybir.AxisListType.X)
                            wb = nwp.tile([P, P], f32, tag="redB")
                            nc.vector.reduce_sum(
                                wb[:], gb[:, ob:ob + kb, :].rearrange(
                                    "p k f -> p f k"),
                                axis=mybir.AxisListType.X)
                            nc.vector.tensor_tensor(
                                out=accn[:], in0=wa[:], in1=wb[:],
                                op=mybir.AluOpType.add)
                        elif ka or kb:
                            sl = (gb[:, oa:oa + ka, :] if ka
                                  else gb[:, ob:ob + kb, :])
                            nc.vector.reduce_sum(
                                accn[:], sl.rearrange("p k f -> p f k"),
                                axis=mybir.AxisListType.X)
                        else:
                            accn = None
                        # + self-loop term dinv_t * u_t (kept on-chip
                        # instead of being gathered)
                        acc1 = nwp.tile([P, P], f32, tag="accT")
                        if accn is None:
                            nc.vector.tensor_copy(acc1[:], tns)
                        else:
                            nc.vector.tensor_tensor(
                                out=acc1[:], in0=accn[:], in1=tns,
                                op=mybir.AluOpType.add)
                        acc2 = nwp.tile([P, P], f32, tag="accs")
                        nc.scalar.activation(acc2[:], acc1[:], AF.Copy,
                                             scale=dinv_sb[:, t:t + 1])
                        pt = ptp.tile([P, P], f32, space="PSUM", tag="tp")
                        nc.tensor.transpose(pt[:], acc2[:], ident[:])
                        nc.scalar.copy(acc_sb[:, t * P:(t + 1) * P], pt[:])

                if skip_reduce:
                    nc.vector.memset(acc_sb[:], 0.0)

                if has_bn:
                    gname = "bng0" if layer == 0 else "bng1"
                    bname = "bnb0" if layer == 0 else "bnb1"
                    ssum = pp.tile([P, 1], f32, tag="ssum")
                    nc.vector.reduce_sum(ssum[:], acc_sb[:, 0:SL],
                                         axis=mybir.AxisListType.X)
                    sq_parts = pp.tile([P, len(chunks)], f32, tag="sqp")
                    for ci, (o, w) in enumerate(chunks):
                        scr = wp.tile([P, 512], f32, tag="u512")
                        nc.scalar.activation(scr[:, :w], acc_sb[:, o:o + w],
                                             AF.Square,
                                             accum_out=sq_parts[:, ci:ci + 1])
                    ssq = pp.tile([P, 1], f32, tag="ssq")
                    nc.vector.reduce_sum(ssq[:], sq_parts[:],
                                         axis=mybir.AxisListType.X)
                    stat_sb = pp.tile([P, 2], f32, tag="stat")
                    nc.vector.tensor_copy(stat_sb[:, 0:1], ssum[:])
                    nc.vector.tensor_copy(stat_sb[:, 1:2], ssq[:])
                    st_out_d = st_out_ds[layer]
                    nc.sync.dma_start(st_in_d[:, :], stat_sb[:])
                    if skip_coll:
                        nc.sync.dma_start(st_out_d[:, :], st_in_d[:, :])
                    else:
                        nc.gpsimd.collective_compute(
                            "AllReduce", mybir.AluOpType.add,
                            replica_groups=[list(range(C))],
                            ins=[st_in_d[:, :].opt()],
                            outs=[st_out_d[:, :].opt()],
                        )
                    stat_g = pp.tile([P, 2], f32, tag="statg")
                    nc.sync.dma_start(stat_g[:], st_out_d[:, :])
                    inv_n = 1.0 / float(n_real_total)
                    mean = pp.tile([P, 1], f32, tag="mean")
                    nc.scalar.mul(mean[:], stat_g[:, 0:1], inv_n)
                    ex2 = pp.tile([P, 1], f32, tag="ex2")
                    nc.scalar.mul(ex2[:], stat_g[:, 1:2], inv_n)
                    m2 = pp.tile([P, 1], f32, tag="m2")
                    nc.scalar.square(m2[:], mean[:])
                    var = pp.tile([P, 1], f32, tag="var")
                    nc.vector.tensor_tensor(out=var[:], in0=ex2[:], in1=m2[:],
                                            op=mybir.AluOpType.subtract)
                    vare = pp.tile([P, 1], f32, tag="vare")
                    nc.vector.tensor_scalar_add(vare[:], var[:], float(EPS))
                    sd = pp.tile([P, 1], f32, tag="sd")
                    nc.scalar.activation(sd[:], vare[:], AF.Sqrt)
                    rs = pp.tile([P, 1], f32, tag="rs")
                    nc.vector.reciprocal(rs[:], sd[:])
                    s_bn = pp.tile([P, 1], f32, tag="sbn")
                    nc.vector.tensor_tensor(out=s_bn[:], in0=rs[:],
                                            in1=v_sb[gname][:, 0:1],
                                            op=mybir.AluOpType.mult)
                    ms = pp.tile([P, 1], f32, tag="ms")
                    nc.vector.tensor_tensor(out=ms[:], in0=mean[:], in1=s_bn[:],
                                            op=mybir.AluOpType.mult)
                    t_bn = pp.tile([P, 1], f32, tag="tbn")
                    nc.vector.tensor_tensor(out=t_bn[:], in0=v_sb[bname][:, 0:1],
                                            in1=ms[:],
                                            op=mybir.AluOpType.subtract)
                    for (o, w) in chunks:
                        nc.scalar.activation(h_sb[:, o:o + w],
                                             acc_sb[:, o:o + w], AF.Relu,
                                             bias=t_bn[:, 0:1],
                                             scale=s_bn[:, 0:1])
                    nc.vector.memset(h_sb[:, REAL:SL], 0.0)
                else:
                    for (o, w) in chunks:
                        nc.scalar.activation(h_sb[:, o:o + w],
                                             acc_sb[:, o:o + w], AF.Identity,
                                             bias=v_sb["cb2"][:, 0:1])

            # ---- post-MLP ----
            for (o, w) in chunks:
                ps = pmm.tile([P, 512], f32, space="PSUM", tag="mm")
                nc.tensor.matmul(ps[:, :w], lhsT=w_sb["pw1"][:],
                                 rhs=h_sb[:, o:o + w], start=True, stop=True)
                t0 = wp.tile([P, 512], f32, tag="u512")
                nc.scalar.activation(t0[:, :w], ps[:, :w], AF.Relu,
                                     bias=v_sb["pb1"][:, 0:1])
                ps2 = pmm.tile([P, 512], f32, space="PSUM", tag="mm")
                nc.tensor.matmul(ps2[:DOUT, :w], lhsT=w_sb["pw2"][:],
                                 rhs=t0[:, :w], start=True, stop=True)
                ot = wp.tile([DOUT, 512], f32, tag="o512")
                nc.scalar.activation(ot[:, :w], ps2[:DOUT, :w], AF.Identity,
                                     bias=v_sb["pb2"][0:DOUT, 0:1])
                nc.sync.dma_start(out_t[:, o:o + w], ot[:, :w])

    nc.compile()
    return nc


# ------------------------------------------------------------------ run -----
def _prepare_in_maps(inputs, meta):
    x = np.asarray(inputs["x"], np.float32)
    SL, REAL, TPC = meta["SL"], meta["REAL"], meta["TPC"]
    xt_perm, dinv_cols, idx_sb = meta["xt_perm"], meta["dinv_cols"], meta["idx_sb"]

    def rep(a):
        return np.ascontiguousarray(a.astype(np.float32))

    def col(a, wdt):
        # pad a [rows<=128, wdt] block up to [128, wdt]
        a = rep(a).reshape(-1, wdt)
        out = np.zeros((P, wdt), np.float32)
        out[:a.shape[0]] = a
        return out

    w_blocks = [
        ("pre_w1", col(inputs["pre_w1"], H)),
        ("pre_w2", col(inputs["pre_w2"], H)),
        ("cw0", col(inputs["conv_w0"], H)),
        ("cw1", col(inputs["conv_w1"], H)),
        ("cw2", col(inputs["conv_w2"], H)),
        ("pw1", col(inputs["post_w1"], H)),
        ("pw2", col(inputs["post_w2"], DOUT)),
        ("pre_b1", col(inputs["pre_b1"].reshape(H, 1), 1)),
        ("pre_b2", col(inputs["pre_b2"].reshape(H, 1), 1)),
        ("cb2", col(inputs["conv_b2"].reshape(H, 1), 1)),
        ("bng0", col(inputs["bn_g0"].reshape(H, 1), 1)),
        ("bnb0", col(inputs["bn_b0"].reshape(H, 1), 1)),
        ("bng1", col(inputs["bn_g1"].reshape(H, 1), 1)),
        ("bnb1", col(inputs["bn_b1"].reshape(H, 1), 1)),
        ("pb1", col(inputs["post_b1"].reshape(H, 1), 1)),
        ("pb2", col(inputs["post_b2"].reshape(DOUT, 1), 1)),
    ]
    wtail = np.concatenate([b for _, b in w_blocks], axis=1)

    in_maps = []
    for c in range(C):
        xc = np.zeros((SL, x.shape[1]), np.float32)
        xc[:REAL] = x[xt_perm[c, :REAL]]
        idx_f32 = np.ascontiguousarray(idx_sb[c]).view(np.float32)
        big = np.concatenate(
            [np.ascontiguousarray(xc.T),
             np.ascontiguousarray(dinv_cols[c]), wtail, idx_f32], axis=1)
        in_maps.append({"big": np.ascontiguousarray(big)})
    return in_maps


def _assemble_output(results, meta, n_nodes):
    SL, REAL = meta["SL"], meta["REAL"]
    xt_perm = meta["xt_perm"]
    out = np.zeros((n_nodes, DOUT), np.float32)
    for c in range(C):
        oc = results[c]["out_t"]          # [DOUT, SL]
        out[xt_perm[c, :REAL]] = oc[:, :REAL].T
    return out


def _install_neff_disk_cache():
    import hashlib
    import shutil

    import concourse.bass2jax as b2j
    import concourse.bass_utils as bu

    if getattr(b2j, "_gcn_neff_cache", False):
        return
    cache_dir = os.environ.get("GCN_NEFF_CACHE", "/tmp/gcn_neff_cache")
    os.makedirs(cache_dir, exist_ok=True)
    orig = bu.compile_bir_kernel

    def cached(bir_json, tmpdir, neff_name="file.neff"):
        h = hashlib.sha256(bir_json if isinstance(bir_json, bytes)
                           else bir_json.encode()).hexdigest()[:24]
        hit = os.path.join(cache_dir, f"{h}.neff")
        dst_dir = os.path.join(tmpdir, "sg00")
        if os.path.exists(hit):
            os.makedirs(dst_dir, exist_ok=True)
            dst = os.path.join(dst_dir, neff_name)
            shutil.copy(hit, dst)
            return dst
        neff = orig(bir_json, tmpdir, neff_name)
        try:
            shutil.copy(neff, hit)
        except OSError:
            pass
        return neff

    b2j.compile_bir_kernel = cached
    bu.compile_bir_kernel = cached
    b2j._gcn_neff_cache = True


def kernel(**inputs):
    from concourse.bass_utils import run_bass_kernel_spmd

    _install_neff_disk_cache()

    edge_index = np.asarray(inputs["edge_index"])
    n_nodes = int(np.asarray(inputs["x"]).shape[0])

    key = (n_nodes, edge_index.shape[1])
    if key not in _cache or os.environ.get("GCN_NO_CACHE"):
        meta = _preprocess(edge_index, n_nodes)
        nc = _build(meta, n_nodes)
        _cache[key] = (meta, nc, edge_index.tobytes())
    meta, nc, eb = _cache[key]
    if eb != edge_index.tobytes():
        meta = _preprocess(edge_index, n_nodes)
        nc = _build(meta, n_nodes)
        _cache[key] = (meta, nc, edge_index.tobytes())

    in_maps = _prepare_in_maps(inputs, meta)
    res = run_bass_kernel_spmd(
        nc, in_maps, core_ids=list(range(C)),
        trace=bool(os.environ.get("GCN_TRACE")),
    )
    out = _assemble_output(res.results, meta, n_nodes)
    if res.exec_time_ns is not None:
        kernel.last_exec_time_ns = res.exec_time_ns
    kernel.last_results = res
    return out


kernel.last_exec_time_ns = None
kernel.last_results = None
